# revision 2
# baseline (speedup 1.0000x reference)
"""DomainCalibratedLoss Trainium2 kernel.

loss = mean_n [ logsumexp_c(x[n,c] + log C[d_n,c]) - (x[n,t_n] + log C[d_n,t_n]) ]

v6 over v5:
  - B matmuls packed 2 tiles per instruction via a block-diagonal rhs:
    lhsT = [6,128] (two tiles' one-hots stacked, K=6), rhs = [6,400]
    (logc twice, block-diagonal), N=400 fits one PSUM bank.  4 matmuls
    per group instead of 8 -> halves PE LDW+dispatch overhead.
  - Optional hybrid add: the first PE_BLOCKS 400-wide blocks get z = B+x
    accumulated ON PE (identity matmul, start=False) and exp reads PSUM;
    the rest use the DVE add into bf16 SBUF.  Balances PE/DVE/ACT.
"""

import sys

sys.path.insert(0, "/opt/trn_rl_repo")

import numpy as np
import ml_dtypes

import concourse.bass as bass
import concourse.bacc as bacc
import concourse.tile as tile
from concourse import mybir
from concourse.bass_utils import run_bass_kernel_spmd
from concourse.masks import make_identity

P = 128          # partitions / points per tile
C = 200          # classes
D = 3            # domains
G = 8            # tiles per group (one x DMA)
NB = 4           # 400-wide blocks per group (2 tiles each)
BW = 2 * C       # block width = 400
BP = 512         # block stride in PSUM (bank)
GPP = 4          # groups per page (one oh DMA)
PT = G * GPP     # tiles per page = 32
N_CORES = 8

PE_BLOCKS = 2    # 400-wide blocks whose z=B+x is accumulated on PE

BF = mybir.dt.bfloat16
NPBF = ml_dtypes.bfloat16

_PROGRAM_CACHE = {}


def build_program(n_pages, reps=1):
    key = (n_pages, reps, PE_BLOCKS)
    if key in _PROGRAM_CACHE:
        return _PROGRAM_CACHE[key]

    T = n_pages * PT
    n_groups = T // G
    s_per = T * P

    nc = bacc.Bacc("TRN2", target_bir_lowering=False, debug=False,
                   num_devices=N_CORES)
    x_in = nc.dram_tensor("x", [n_groups * P, G * C], BF,
                          kind="ExternalInput").ap()
    # one-hot stacked pairs: per page [6, GPP*NB, P]
    oh_in = nc.dram_tensor("oh", [n_pages * 2 * D, GPP * NB * P], BF,
                           kind="ExternalInput").ap()
    sub_in = nc.dram_tensor("subt", [P, T], mybir.dt.float32,
                            kind="ExternalInput").ap()
    # block-diagonal logc: [6, 400]
    logc_in = nc.dram_tensor("logc2", [2 * D, BW], BF,
                             kind="ExternalInput").ap()
    r_out = nc.dram_tensor("r", [P, 1], mybir.dt.float32,
                           kind="ExternalOutput").ap()

    with tile.TileContext(nc) as tc:
        with (
            tc.tile_pool(name="singles", bufs=1) as singles,
            tc.tile_pool(name="xp", bufs=6) as xp,
            tc.tile_pool(name="ohp", bufs=3) as ohp,
            tc.tile_pool(name="ebp", bufs=3) as ebp,
            tc.tile_pool(name="zbp", bufs=3) as zbp,
            tc.tile_pool(name="ps", bufs=2, space="PSUM") as ps,
        ):
            logc2 = singles.tile([2 * D, BW], BF)
            nc.sync.dma_start(out=logc2[:], in_=logc_in[:])
            sub_all = singles.tile([P, T], mybir.dt.float32)
            nc.sync.dma_start(out=sub_all[:], in_=sub_in[:])
            S_all = singles.tile([P, T], mybir.dt.float32)
            L_all = singles.tile([P, T], mybir.dt.float32)
            ident = None
            if PE_BLOCKS > 0:
                ident = singles.tile([P, P], BF)
                make_identity(nc, ident)

            def one_pass():
                pending = []

                def flush_reduce():
                    eb_p, lo = pending.pop(0)
                    nc.vector.tensor_reduce(
                        out=S_all[:, lo:lo + G],
                        in_=eb_p[:].rearrange("p (g c) -> p g c", c=C),
                        axis=mybir.AxisListType.X,
                        op=mybir.AluOpType.add)

                for pg in range(n_pages):
                    ohg = ohp.tile([2 * D, GPP * NB, P], BF, tag="oh")
                    nc.sync.dma_start(
                        out=ohg[:],
                        in_=oh_in[bass.ts(pg, 2 * D), :].rearrange(
                            "d (k p) -> d k p", p=P))
                    for g4 in range(GPP):
                        gi = pg * GPP + g4
                        xg = xp.tile([P, G * C], BF, tag="x")
                        nc.sync.dma_start(out=xg[:],
                                          in_=x_in[bass.ts(gi, P), :])
                        xv = xg[:].rearrange("p (k q) -> p k q", q=BW)
                        bz = ps.tile([P, NB, BP], mybir.dt.float32, tag="bz")
                        for k in range(NB):
                            nc.tensor.matmul(bz[:, k, 0:BW],
                                             lhsT=ohg[:, g4 * NB + k, :],
                                             rhs=logc2[:],
                                             start=True,
                                             stop=(k >= PE_BLOCKS))
                        for k in range(PE_BLOCKS):
                            nc.tensor.matmul(bz[:, k, 0:BW],
                                             lhsT=ident[:],
                                             rhs=xv[:, k, :],
                                             start=False, stop=True)
                        eb = ebp.tile([P, G * C], BF, tag="e")
                        ev = eb[:].rearrange("p (k q) -> p k q", q=BW)
                        if PE_BLOCKS > 0:
                            nc.scalar.activation(
                                ev[:, 0:PE_BLOCKS, :],
                                bz[:, 0:PE_BLOCKS, 0:BW],
                                mybir.ActivationFunctionType.Exp)
                        if PE_BLOCKS < NB:
                            zb = zbp.tile([P, (NB - PE_BLOCKS) * BW], BF,
                                          tag="z")
                            zv = zb[:].rearrange("p (k q) -> p k q", q=BW)
                            nc.vector.tensor_tensor(
                                out=zv[:],
                                in0=xv[:, PE_BLOCKS:NB, :],
                                in1=bz[:, PE_BLOCKS:NB, 0:BW],
                                op=mybir.AluOpType.add)
                            nc.scalar.activation(
                                ev[:, PE_BLOCKS:NB, :], zv[:],
                                mybir.ActivationFunctionType.Exp)
                        pending.append((eb, gi * G))
                        if len(pending) > 1:
                            flush_reduce()
                while pending:
                    flush_reduce()
                # epilogue: L = ln(S) - sub
                nc.scalar.activation(L_all[:], S_all[:],
                                     mybir.ActivationFunctionType.Ln)
                nc.vector.tensor_tensor(out=L_all[:], in0=L_all[:],
                                        in1=sub_all[:],
                                        op=mybir.AluOpType.subtract)

            if reps == 1:
                one_pass()
            else:
                with tc.For_i(0, reps):
                    one_pass()

            r = singles.tile([P, 1], mybir.dt.float32)
            nc.vector.tensor_reduce(out=r[:], in_=L_all[:],
                                    axis=mybir.AxisListType.X,
                                    op=mybir.AluOpType.add)
            nc.sync.dma_start(out=r_out[:], in_=r[:])

    nc.compile()
    _PROGRAM_CACHE[key] = nc
    return nc


def _host_prep(inputs, targets, domains, domain_counts, n_pages):
    """Build the per-core input maps (host-side sharding/marshalling)."""
    n = inputs.shape[0]
    T = n_pages * PT
    s_per = T * P
    n_groups = T // G
    n_pad = N_CORES * s_per

    logc = np.log(domain_counts.astype(np.float32)).astype(np.float32)
    tgt = targets.astype(np.int64).reshape(-1)
    dom = domains.astype(np.int64).reshape(-1)

    sub = np.empty(n_pad, dtype=np.float32)
    sub[:n] = inputs[np.arange(n), tgt] + logc[dom, tgt]
    sub[n:] = np.float32(np.log(float(C)))

    dom_pad = np.full(n_pad, -1, dtype=np.int64)
    dom_pad[:n] = dom

    # block-diagonal logc [6, 400]
    logc2 = np.zeros((2 * D, BW), dtype=np.float32)
    logc2[0:D, 0:C] = logc
    logc2[D:2 * D, C:2 * C] = logc

    in_maps = []
    for c in range(N_CORES):
        lo = c * s_per
        x_c = np.zeros((s_per, C), dtype=np.float32)
        n_real = max(0, min(s_per, n - lo))
        if n_real:
            x_c[:n_real] = inputs[lo:lo + n_real]
        x_m = np.ascontiguousarray(
            x_c.reshape(n_groups, G, P, C).transpose(0, 2, 1, 3)
        ).reshape(n_groups * P, G * C).astype(NPBF)
        dom_c = dom_pad[lo:lo + s_per]
        # oh[pg, r'*3+d, (g4*NB+k), p] = dom[tile g4*8+2k+r', p] == d
        dom_t = dom_c.reshape(n_pages, PT, P)       # [pg, tile, p]
        # tile index = (g4*NB + k)*2 + r' where within page; map tiles ->
        # [pg, pair, r', p] with pair = tile//2, r' = tile%2
        dom_pair = dom_t.reshape(n_pages, PT // 2, 2, P)
        oh = (dom_pair[:, None, :, :, :]
              == np.arange(D).reshape(1, D, 1, 1, 1))   # [pg, d, pair, r', p]
        # -> [pg, r'*D+d, pair, p]
        oh = oh.transpose(0, 3, 1, 2, 4).astype(NPBF)   # [pg, r', d, pair, p]
        oh = oh.reshape(n_pages, 2 * D, (PT // 2) * P)
        sub_c = np.ascontiguousarray(
            sub[lo:lo + s_per].reshape(T, P).T)
        in_maps.append({
            "x": x_m,
            "oh": np.ascontiguousarray(oh.reshape(n_pages * 2 * D,
                                                  (PT // 2) * P)),
            "subt": sub_c,
            "logc2": logc2.astype(NPBF),
        })
    return in_maps


def kernel(inputs, targets, domains, domain_counts):
    inputs = np.asarray(inputs, dtype=np.float32)
    targets_np = np.asarray(targets).reshape(-1)
    domains_np = np.asarray(domains).reshape(-1)
    counts = np.asarray(domain_counts, dtype=np.float32)

    n = inputs.shape[0]
    n_pages = -(-n // (N_CORES * PT * P))            # ceil -> 31 for N=1M

    nc = build_program(n_pages, reps=1)
    in_maps = _host_prep(inputs, targets_np, domains_np, counts, n_pages)
    res = run_bass_kernel_spmd(nc, in_maps, list(range(N_CORES)))

    total = 0.0
    for r in res.results:
        total += r["r"].astype(np.float64).sum()
    n_valid = int((targets_np != 255).sum())
    return np.float32(total / n_valid)


# revision 3
# speedup vs baseline: 1.6566x; 1.6566x over previous
"""DomainCalibratedLoss Trainium2 kernel.

loss = mean_n [ logsumexp_c(x[n,c] + log C[d_n,c]) - (x[n,t_n] + log C[d_n,t_n]) ]

v6 over v5:
  - B matmuls packed 2 tiles per instruction via a block-diagonal rhs:
    lhsT = [6,128] (two tiles' one-hots stacked, K=6), rhs = [6,400]
    (logc twice, block-diagonal), N=400 fits one PSUM bank.  4 matmuls
    per group instead of 8 -> halves PE LDW+dispatch overhead.
  - Optional hybrid add: the first PE_BLOCKS 400-wide blocks get z = B+x
    accumulated ON PE (identity matmul, start=False) and exp reads PSUM;
    the rest use the DVE add into bf16 SBUF.  Balances PE/DVE/ACT.
"""

import sys

sys.path.insert(0, "/opt/trn_rl_repo")

import numpy as np
import ml_dtypes

import concourse.bass as bass
import concourse.bacc as bacc
import concourse.tile as tile
from concourse import mybir
from concourse.bass_utils import run_bass_kernel_spmd
from concourse.masks import make_identity

P = 128          # partitions / points per tile
C = 200          # classes
D = 3            # domains
G = 8            # tiles per group (one x DMA)
NB = 4           # 400-wide blocks per group (2 tiles each)
BW = 2 * C       # block width = 400
BP = 512         # block stride in PSUM (bank)
GPP = 4          # groups per page (one oh DMA)
PT = G * GPP     # tiles per page = 32
N_CORES = 8

PE_BLOCKS = 2    # 400-wide blocks whose z=B+x is accumulated on PE

BF = mybir.dt.bfloat16
NPBF = ml_dtypes.bfloat16

_PROGRAM_CACHE = {}


def build_program(n_pages, reps=1):
    key = (n_pages, reps, PE_BLOCKS)
    if key in _PROGRAM_CACHE:
        return _PROGRAM_CACHE[key]

    T = n_pages * PT
    n_groups = T // G
    s_per = T * P

    nc = bacc.Bacc("TRN2", target_bir_lowering=False, debug=False,
                   num_devices=N_CORES)
    x_in = nc.dram_tensor("x", [n_groups * P, G * C], BF,
                          kind="ExternalInput").ap()
    # one-hot stacked pairs: per page [6, GPP*NB, P]
    oh_in = nc.dram_tensor("oh", [n_pages * 2 * D, GPP * NB * P], BF,
                           kind="ExternalInput").ap()
    sub_in = nc.dram_tensor("subt", [P, T], mybir.dt.float32,
                            kind="ExternalInput").ap()
    # block-diagonal logc: [6, 400]
    logc_in = nc.dram_tensor("logc2", [2 * D, BW], BF,
                             kind="ExternalInput").ap()
    r_out = nc.dram_tensor("r", [P, 1], mybir.dt.float32,
                           kind="ExternalOutput").ap()

    with tile.TileContext(nc) as tc:
        with (
            tc.tile_pool(name="singles", bufs=1) as singles,
            tc.tile_pool(name="xp", bufs=6) as xp,
            tc.tile_pool(name="ohp", bufs=3) as ohp,
            tc.tile_pool(name="ebp", bufs=3) as ebp,
            tc.tile_pool(name="zbp", bufs=3) as zbp,
            tc.tile_pool(name="psA", bufs=2, space="PSUM") as psA,
            tc.tile_pool(name="psB", bufs=2, space="PSUM") as psB,
        ):
            logc2 = singles.tile([2 * D, BW], BF)
            nc.sync.dma_start(out=logc2[:], in_=logc_in[:])
            sub_all = singles.tile([P, T], mybir.dt.float32)
            nc.sync.dma_start(out=sub_all[:], in_=sub_in[:])
            S_all = singles.tile([P, T], mybir.dt.float32)
            L_all = singles.tile([P, T], mybir.dt.float32)
            ident = None
            if PE_BLOCKS > 0:
                ident = singles.tile([P, P], BF)
                make_identity(nc, ident)

            def one_pass():
                pending = []

                def flush_reduce():
                    eb_p, lo = pending.pop(0)
                    nc.vector.tensor_reduce(
                        out=S_all[:, lo:lo + G],
                        in_=eb_p[:].rearrange("p (g c) -> p g c", c=C),
                        axis=mybir.AxisListType.X,
                        op=mybir.AluOpType.add)

                for pg in range(n_pages):
                    ohg = ohp.tile([2 * D, GPP * NB, P], BF, tag="oh")
                    nc.sync.dma_start(
                        out=ohg[:],
                        in_=oh_in[bass.ts(pg, 2 * D), :].rearrange(
                            "d (k p) -> d k p", p=P))
                    for g4 in range(GPP):
                        gi = pg * GPP + g4
                        xg = xp.tile([P, G * C], BF, tag="x")
                        nc.sync.dma_start(out=xg[:],
                                          in_=x_in[bass.ts(gi, P), :])
                        xv = xg[:].rearrange("p (k q) -> p k q", q=BW)
                        bzA = psA.tile([P, PE_BLOCKS, BP],
                                       mybir.dt.float32, tag="bzA")
                        bzB = psB.tile([P, NB - PE_BLOCKS, BP],
                                       mybir.dt.float32, tag="bzB")
                        for k in range(NB):
                            dst = (bzA[:, k, 0:BW] if k < PE_BLOCKS
                                   else bzB[:, k - PE_BLOCKS, 0:BW])
                            nc.tensor.matmul(dst,
                                             lhsT=ohg[:, g4 * NB + k, :],
                                             rhs=logc2[:],
                                             start=True,
                                             stop=(k >= PE_BLOCKS))
                        for k in range(PE_BLOCKS):
                            nc.tensor.matmul(bzA[:, k, 0:BW],
                                             lhsT=ident[:],
                                             rhs=xv[:, k, :],
                                             start=False, stop=True)
                        eb = ebp.tile([P, G * C], BF, tag="e")
                        ev = eb[:].rearrange("p (k q) -> p k q", q=BW)
                        if PE_BLOCKS > 0:
                            nc.scalar.activation(
                                ev[:, 0:PE_BLOCKS, :],
                                bzA[:, :, 0:BW],
                                mybir.ActivationFunctionType.Exp)
                        if PE_BLOCKS < NB:
                            zb = zbp.tile([P, (NB - PE_BLOCKS) * BW], BF,
                                          tag="z")
                            zv = zb[:].rearrange("p (k q) -> p k q", q=BW)
                            nc.vector.tensor_tensor(
                                out=zv[:],
                                in0=xv[:, PE_BLOCKS:NB, :],
                                in1=bzB[:, :, 0:BW],
                                op=mybir.AluOpType.add)
                            nc.scalar.activation(
                                ev[:, PE_BLOCKS:NB, :], zv[:],
                                mybir.ActivationFunctionType.Exp)
                        pending.append((eb, gi * G))
                        if len(pending) > 1:
                            flush_reduce()
                while pending:
                    flush_reduce()
                # epilogue: L = ln(S) - sub
                nc.scalar.activation(L_all[:], S_all[:],
                                     mybir.ActivationFunctionType.Ln)
                nc.vector.tensor_tensor(out=L_all[:], in0=L_all[:],
                                        in1=sub_all[:],
                                        op=mybir.AluOpType.subtract)

            if reps == 1:
                one_pass()
            else:
                with tc.For_i(0, reps):
                    one_pass()

            r = singles.tile([P, 1], mybir.dt.float32)
            nc.vector.tensor_reduce(out=r[:], in_=L_all[:],
                                    axis=mybir.AxisListType.X,
                                    op=mybir.AluOpType.add)
            nc.sync.dma_start(out=r_out[:], in_=r[:])

    nc.compile()
    _PROGRAM_CACHE[key] = nc
    return nc


def _host_prep(inputs, targets, domains, domain_counts, n_pages):
    """Build the per-core input maps (host-side sharding/marshalling)."""
    n = inputs.shape[0]
    T = n_pages * PT
    s_per = T * P
    n_groups = T // G
    n_pad = N_CORES * s_per

    logc = np.log(domain_counts.astype(np.float32)).astype(np.float32)
    tgt = targets.astype(np.int64).reshape(-1)
    dom = domains.astype(np.int64).reshape(-1)

    sub = np.empty(n_pad, dtype=np.float32)
    sub[:n] = inputs[np.arange(n), tgt] + logc[dom, tgt]
    sub[n:] = np.float32(np.log(float(C)))

    dom_pad = np.full(n_pad, -1, dtype=np.int64)
    dom_pad[:n] = dom

    # block-diagonal logc [6, 400]
    logc2 = np.zeros((2 * D, BW), dtype=np.float32)
    logc2[0:D, 0:C] = logc
    logc2[D:2 * D, C:2 * C] = logc

    in_maps = []
    for c in range(N_CORES):
        lo = c * s_per
        x_c = np.zeros((s_per, C), dtype=np.float32)
        n_real = max(0, min(s_per, n - lo))
        if n_real:
            x_c[:n_real] = inputs[lo:lo + n_real]
        x_m = np.ascontiguousarray(
            x_c.reshape(n_groups, G, P, C).transpose(0, 2, 1, 3)
        ).reshape(n_groups * P, G * C).astype(NPBF)
        dom_c = dom_pad[lo:lo + s_per]
        # oh[pg, r'*3+d, (g4*NB+k), p] = dom[tile g4*8+2k+r', p] == d
        dom_t = dom_c.reshape(n_pages, PT, P)       # [pg, tile, p]
        # tile index = (g4*NB + k)*2 + r' where within page; map tiles ->
        # [pg, pair, r', p] with pair = tile//2, r' = tile%2
        dom_pair = dom_t.reshape(n_pages, PT // 2, 2, P)
        oh = (dom_pair[:, None, :, :, :]
              == np.arange(D).reshape(1, D, 1, 1, 1))   # [pg, d, pair, r', p]
        # -> [pg, r'*D+d, pair, p]
        oh = oh.transpose(0, 3, 1, 2, 4).astype(NPBF)   # [pg, r', d, pair, p]
        oh = oh.reshape(n_pages, 2 * D, (PT // 2) * P)
        sub_c = np.ascontiguousarray(
            sub[lo:lo + s_per].reshape(T, P).T)
        in_maps.append({
            "x": x_m,
            "oh": np.ascontiguousarray(oh.reshape(n_pages * 2 * D,
                                                  (PT // 2) * P)),
            "subt": sub_c,
            "logc2": logc2.astype(NPBF),
        })
    return in_maps


def kernel(inputs, targets, domains, domain_counts):
    inputs = np.asarray(inputs, dtype=np.float32)
    targets_np = np.asarray(targets).reshape(-1)
    domains_np = np.asarray(domains).reshape(-1)
    counts = np.asarray(domain_counts, dtype=np.float32)

    n = inputs.shape[0]
    n_pages = -(-n // (N_CORES * PT * P))            # ceil -> 31 for N=1M

    nc = build_program(n_pages, reps=1)
    in_maps = _host_prep(inputs, targets_np, domains_np, counts, n_pages)
    res = run_bass_kernel_spmd(nc, in_maps, list(range(N_CORES)))

    total = 0.0
    for r in res.results:
        total += r["r"].astype(np.float64).sum()
    n_valid = int((targets_np != 255).sum())
    return np.float32(total / n_valid)
